# revision 25
# baseline (speedup 1.0000x reference)
"""DeepPoly affine transformer — Trainium2 Bass kernel (8 NeuronCores).

Math: with A = |W|, and beta,lmbda >= 0 (so |Ml|,|Mu| are linear in W,|W|):
  lower     = W@v1 + A@v2 + b         v1 = (l+u)/2,  v2 = (l-u)/2
  upper     = W@v1 - A@v2 + b
  new_lower = W@a  + A@c  + b         s1 = (beta+lmbda)/2, s2 = (beta-lmbda)/2
  new_upper = W@a  - A@c  + b         a  = (s1*(l0+u0) + s2*(l0-u0) + mu)/2
                                      c  = (s2*(l0+u0) + s1*(l0-u0) - mu)/2
  out_lower = max(lower, new_lower);  out_upper = min(upper, new_upper)

Each core gets a 512-row slice of W (row-sharded over n_out), host-transposed
to W^T and cast to fp16 (10-bit mantissa; W ~ N(0, 0.05^2) fits comfortably),
stored partition-major [128, 64, 512] so every DMA reads long contiguous runs.
The PE streams W^T and |W^T| (2-byte operands stream at full rate, 4-byte at
half rate) against 2 stationary vectors each, accumulating in fp32 PSUM:
yz = [W@v1, W@a | A@v2, A@c]. The O(n_out) final combine runs on host.
"""

import numpy as np

import concourse.mybir as mybir
import concourse.tile as tile
from concourse import bacc
from concourse.bass_utils import run_bass_kernel_spmd

N_OUT, N_IN = 4096, 8192
NCORES = 8
S = N_OUT // NCORES  # 512 output rows per core
P = 128
KT = N_IN // P       # 64 k-tiles of 128
KSUP = 8             # k-tiles per DMA supertile (1 MiB fp16 loads)
F32 = mybir.dt.float32
F16 = mybir.dt.float16

_CACHE = {}

NB = 6          # supertile ring depth
SUPS = [2, 6] + [KSUP] * ((KT - 8) // KSUP)   # staggered supertile sizes


def _build_nc_raw():
    """Raw bass version: manual semaphores, no Tile entry/exit barriers
    (the Tile epilogue alone costs ~8 us of drain + sem-clear butterfly).

    DMA completion uses one semaphore per ring slot: a cumulative count on a
    shared sem is skew-unsafe (a lagging SDMA engine's missing inc can be
    masked by a faster engine's incs for a LATER dma). With per-slot sems the
    only other writers of s_w[k] are w_{i±4*n}, and w_{i+4} cannot even be
    issued until the consumers of w_i signalled s_pe — so each wait threshold
    is exact and unpollutable."""
    nc = bacc.Bacc("TRN2", target_bir_lowering=False, debug=False,
                   num_devices=NCORES)
    wt = nc.dram_tensor("wt", [P, KT, S], F16, kind="ExternalInput").ap()
    vecs = nc.dram_tensor("vecs", [P, KT, 4], F16, kind="ExternalInput").ap()
    yz = nc.dram_tensor("yz", [2, 2 * S], F32, kind="ExternalOutput").ap()
    I16 = mybir.dt.int16
    nsup = len(SUPS)
    starts = np.cumsum([0] + SUPS).tolist()

    with (
        nc.sbuf_tensor("w_sb", [P, NB, KSUP, S], F16) as w_sb_t,
        nc.sbuf_tensor("a_sb", [P, NB, KSUP, S], F16) as a_sb_t,
        nc.sbuf_tensor("v_sb", [P, KT, 4], F16) as v_sb_t,
        nc.sbuf_tensor("out_sb", [2, 2 * S], F32) as out_sb_t,
        nc.psum_tensor("psum_y", [2, S], F32) as psum_y_t,
        nc.psum_tensor("psum_z", [2, S], F32) as psum_z_t,
        nc.semaphore("s_w0") as s_w0,
        nc.semaphore("s_w1") as s_w1,
        nc.semaphore("s_w2") as s_w2,
        nc.semaphore("s_w3") as s_w3,
        nc.semaphore("s_w4") as s_w4,
        nc.semaphore("s_w5") as s_w5,
        nc.semaphore("s_v") as s_v,
        nc.semaphore("s_abs") as s_abs,
        nc.semaphore("s_pe") as s_pe,
        nc.semaphore("s_copy") as s_copy,
        nc.Block() as block,
    ):
        w_sb = w_sb_t.ap()
        a_sb = a_sb_t.ap()
        v_sb = v_sb_t.ap()
        out_sb = out_sb_t.ap()
        psum_y = psum_y_t.ap()
        psum_z = psum_z_t.ap()
        s_w = [s_w0, s_w1, s_w2, s_w3, s_w4, s_w5]

        def wsem(i):
            return s_w[i % NB], 16 * (i // NB + 1)

        @block.sync
        def _(sync):
            for i, ksup in enumerate(SUPS):
                if i >= NB:
                    # slot reuse: PE finished supertile i-NB (implies abs too)
                    sync.wait_ge(s_pe, i - NB + 1)
                sem, _ = wsem(i)
                sync.dma_start(
                    w_sb[:, i % NB, :ksup, :],
                    wt[:, starts[i]:starts[i] + ksup, :],
                ).then_inc(sem, 16)
                if i == 0:
                    sync.dma_start(v_sb[:], vecs).then_inc(s_v, 16)
            sync.wait_ge(s_copy, 2)
            # every DMA needs a sem update (walrus asserts otherwise)
            sync.dma_start(yz, out_sb[:]).then_inc(s_v, 16)

        @block.vector
        def _(vector):
            for i, ksup in enumerate(SUPS):
                sem, val = wsem(i)
                vector.wait_ge(sem, val)
                if i >= NB:
                    vector.wait_ge(s_pe, i - NB + 1)
                # two abs chunks per supertile so Z matmuls start earlier
                h = max(1, ksup // 2)
                for c0, c1 in ((0, h), (h, ksup)):
                    if c1 > c0:
                        vector.tensor_scalar(
                            a_sb[:, i % NB, c0:c1, :].bitcast(I16),
                            w_sb[:, i % NB, c0:c1, :].bitcast(I16),
                            0x7FFF, None, mybir.AluOpType.bitwise_and,
                        ).then_inc(s_abs, 1)
                    else:
                        vector.nop().then_inc(s_abs, 1)
            vector.wait_ge(s_pe, nsup)
            vector.tensor_copy(out=out_sb[:, 0:S], in_=psum_y[:])
            vector.tensor_copy(
                out=out_sb[:, S:2 * S], in_=psum_z[:]).then_inc(s_copy, 2)

        @block.tensor
        def _(tensor):
            tensor.wait_ge(s_v, 16)
            for i, ksup in enumerate(SUPS):
                sem, val = wsem(i)
                tensor.wait_ge(sem, val)
                h = max(1, ksup // 2)
                for j in range(ksup):
                    ki = starts[i] + j
                    tensor.matmul(
                        psum_y[:],
                        v_sb[:, ki, 0:2],
                        w_sb[:, i % NB, j, :],
                        start=(ki == 0), stop=(ki == KT - 1))
                    if j == 0:
                        tensor.wait_ge(s_abs, 2 * i + 1)
                    elif j == h:
                        tensor.wait_ge(s_abs, 2 * i + 2)
                    mm = tensor.matmul(
                        psum_z[:],
                        v_sb[:, ki, 2:4],
                        a_sb[:, i % NB, j, :],
                        start=(ki == 0), stop=(ki == KT - 1))
                mm.then_inc(s_pe, 1)

    nc.compile()
    return nc


def _build_nc():
    nc = bacc.Bacc("TRN2", target_bir_lowering=False, debug=False,
                   num_devices=NCORES)
    # partition-major: wt[p, ki, n] = W^T[ki*128 + p, n]
    wt = nc.dram_tensor("wt", [P, KT, S], F16, kind="ExternalInput").ap()
    vecs = nc.dram_tensor("vecs", [P, KT, 4], F16, kind="ExternalInput").ap()
    yz = nc.dram_tensor("yz", [2, 2 * S], F32, kind="ExternalOutput").ap()

    # first supertiles are small so the PE starts early; rest amortize DMA
    sups = [2, 6] + [KSUP] * ((KT - 8) // KSUP)
    assert sum(sups) == KT

    with tile.TileContext(nc) as tc:
        with (
            tc.tile_pool(name="wp", bufs=5) as wpool,
            tc.tile_pool(name="apool", bufs=5) as apool,
            tc.tile_pool(name="vp", bufs=1) as vpool,
            tc.tile_pool(name="op", bufs=1) as opool,
            tc.tile_pool(name="ps", bufs=1, space="PSUM") as pspool,
            tc.tile_pool(name="wu", bufs=1) as wupool,
            tc.tile_pool(name="wups", bufs=1, space="PSUM") as wupspool,
        ):
            # Warm-up: ~9 dummy matmuls on scratch data run while the first
            # weight DMAs are in flight, flipping the PE HAM clock gate from
            # 1.2 to 2.4 GHz (~3.4us of sustained PE activity) before the
            # real matmul stream starts.
            wu_sb = wupool.tile([P, S], F16)
            nc.vector.memset(wu_sb[:], 0.0)
            wu_ps = wupspool.tile([2, S], F32)
            for _ in range(9):
                nc.tensor.matmul(
                    wu_ps[:], wu_sb[:, 0:2], wu_sb[:],
                    start=True, stop=True)

            v_sb = vpool.tile([P, KT, 4], F16)
            nc.sync.dma_start(v_sb[:], vecs)

            psum_y = pspool.tile([2, S], F32, tag="Y")
            # Z accumulates at PSUM base partition 32 so its matmuls land on
            # a different PE column-group and run concurrently with Y's.
            psum_z = pspool.tile([34, S], F32, tag="Z", name="psum_z")[32:34]

            ki = 0
            for si, ksup in enumerate(sups):
                w_sb = wpool.tile([P, KSUP, S], F16, tag="w", name="w_sb")[:, :ksup, :]
                # alternate the two HWDGE rings (SP / ACT sequencers)
                dma_eng = nc.sync if si % 2 == 0 else nc.scalar
                dma_eng.dma_start(w_sb[:], wt[:, ki:ki + ksup, :])
                a_sb = apool.tile([P, KSUP, S], F16, tag="a", name="a_sb")[:, :ksup, :]
                # fp16 abs = clear the sign bit (abs_max isn't a valid
                # TensorScalar ALU op)
                nc.vector.tensor_scalar(
                    a_sb.bitcast(mybir.dt.int16), w_sb.bitcast(mybir.dt.int16),
                    0x7FFF, None, mybir.AluOpType.bitwise_and)
                for j in range(ksup):
                    nc.tensor.matmul(
                        psum_y[:],
                        v_sb[:, ki, 0:2],
                        w_sb[:, j, :],
                        start=(ki == 0), stop=(ki == KT - 1),
                        tile_position=(0, 0))
                    nc.tensor.matmul(
                        psum_z[:],
                        v_sb[:, ki, 2:4],
                        a_sb[:, j, :],
                        start=(ki == 0), stop=(ki == KT - 1),
                        tile_position=(0, 32))
                    ki += 1

            # pack along free dim: row0 = [y1 | z2], row1 = [ya | zc]
            out_sb = opool.tile([2, 2 * S], F32)
            nc.scalar.copy(out=out_sb[:, 0:S], in_=psum_y[:])
            nc.vector.tensor_copy(out=out_sb[:, S:2 * S], in_=psum_z[:])
            nc.sync.dma_start(yz, out_sb[:])

    nc.compile()
    return nc


USE_RAW = False


def get_nc():
    if "nc" not in _CACHE:
        _CACHE["nc"] = _build_nc_raw() if USE_RAW else _build_nc()
    return _CACHE["nc"]


def _host_vectors(bounds, bounds0, beta, lmbda, mu):
    l, u = bounds[0].astype(np.float64), bounds[1].astype(np.float64)
    l0, u0 = bounds0[0].astype(np.float64), bounds0[1].astype(np.float64)
    beta = beta.astype(np.float64)
    lmbda = lmbda.astype(np.float64)
    mu = mu.astype(np.float64)
    s1 = (beta + lmbda) / 2
    s2 = (beta - lmbda) / 2
    v1 = (l + u) / 2
    v2 = (l - u) / 2
    a = (s1 * (l0 + u0) + s2 * (l0 - u0) + mu) / 2
    c = (s2 * (l0 + u0) + s1 * (l0 - u0) - mu) / 2
    vecs = np.stack([v1, a, v2, c], axis=1)              # [N_IN, 4]
    return np.ascontiguousarray(
        vecs.reshape(KT, P, 4).transpose(1, 0, 2)).astype(np.float16)


def build_in_maps(weight, bounds, bounds0, beta, lmbda, mu):
    vecs = _host_vectors(bounds, bounds0, beta, lmbda, mu)
    in_maps = []
    for i in range(NCORES):
        wt = weight[i * S:(i + 1) * S].T                 # [N_IN, S] view
        wt = np.ascontiguousarray(
            wt.reshape(KT, P, S).transpose(1, 0, 2)).astype(np.float16)
        in_maps.append({"wt": wt, "vecs": vecs})
    return in_maps


def kernel(weight, bias, bounds, bounds0, beta, lmbda, mu):
    nc = get_nc()
    in_maps = build_in_maps(weight, bounds, bounds0, beta, lmbda, mu)
    res = run_bass_kernel_spmd(nc, in_maps, core_ids=list(range(NCORES)))

    lower = np.empty(N_OUT, np.float32)
    upper = np.empty(N_OUT, np.float32)
    for i in range(NCORES):
        yz = res.results[i]["yz"]
        y1, z2 = yz[0, :S], yz[0, S:]
        ya, zc = yz[1, :S], yz[1, S:]
        b = bias[i * S:(i + 1) * S]
        lower[i * S:(i + 1) * S] = np.maximum(y1 + z2, ya + zc) + b
        upper[i * S:(i + 1) * S] = np.minimum(y1 - z2, ya - zc) + b
    return np.stack([lower, upper], axis=0)


# revision 26
# speedup vs baseline: 1.0905x; 1.0905x over previous
"""DeepPoly affine transformer — Trainium2 Bass kernel (8 NeuronCores).

Math: with A = |W|, and beta,lmbda >= 0 (so |Ml|,|Mu| are linear in W,|W|):
  lower     = W@v1 + A@v2 + b         v1 = (l+u)/2,  v2 = (l-u)/2
  upper     = W@v1 - A@v2 + b
  new_lower = W@a  + A@c  + b         s1 = (beta+lmbda)/2, s2 = (beta-lmbda)/2
  new_upper = W@a  - A@c  + b         a  = (s1*(l0+u0) + s2*(l0-u0) + mu)/2
                                      c  = (s2*(l0+u0) + s1*(l0-u0) - mu)/2
  out_lower = max(lower, new_lower);  out_upper = min(upper, new_upper)

Each core gets a 512-row slice of W (row-sharded over n_out), host-transposed
to W^T and cast to fp16 (10-bit mantissa; W ~ N(0, 0.05^2) fits comfortably),
stored partition-major [128, 64, 512] so every DMA reads long contiguous runs.
The PE streams W^T and |W^T| (2-byte operands stream at full rate, 4-byte at
half rate) against 2 stationary vectors each, accumulating in fp32 PSUM:
yz = [W@v1, W@a | A@v2, A@c]. The O(n_out) final combine runs on host.
"""

import numpy as np

import concourse.mybir as mybir
import concourse.tile as tile
from concourse import bacc
from concourse.bass_utils import run_bass_kernel_spmd

N_OUT, N_IN = 4096, 8192
NCORES = 8
S = N_OUT // NCORES  # 512 output rows per core
P = 128
KT = N_IN // P       # 64 k-tiles of 128
KSUP = 8             # k-tiles per DMA supertile (1 MiB fp16 loads)
F32 = mybir.dt.float32
F16 = mybir.dt.float16

_CACHE = {}

NB = 6          # supertile ring depth
SUPS = [2, 6] + [KSUP] * ((KT - 8) // KSUP)   # staggered supertile sizes


def _build_nc_raw():
    """Raw bass version: manual semaphores, no Tile entry/exit barriers
    (the Tile epilogue alone costs ~8 us of drain + sem-clear butterfly).

    DMA completion uses one semaphore per ring slot: a cumulative count on a
    shared sem is skew-unsafe (a lagging SDMA engine's missing inc can be
    masked by a faster engine's incs for a LATER dma). With per-slot sems the
    only other writers of s_w[k] are w_{i±4*n}, and w_{i+4} cannot even be
    issued until the consumers of w_i signalled s_pe — so each wait threshold
    is exact and unpollutable."""
    nc = bacc.Bacc("TRN2", target_bir_lowering=False, debug=False,
                   num_devices=NCORES)
    wt = nc.dram_tensor("wt", [P, KT, S], F16, kind="ExternalInput").ap()
    vecs = nc.dram_tensor("vecs", [P, KT, 4], F16, kind="ExternalInput").ap()
    yz = nc.dram_tensor("yz", [2, 2 * S], F32, kind="ExternalOutput").ap()
    I16 = mybir.dt.int16
    nsup = len(SUPS)
    starts = np.cumsum([0] + SUPS).tolist()

    with (
        nc.sbuf_tensor("w_sb", [P, NB, KSUP, S], F16) as w_sb_t,
        nc.sbuf_tensor("a_sb", [P, NB, KSUP, S], F16) as a_sb_t,
        nc.sbuf_tensor("v_sb", [P, KT, 4], F16) as v_sb_t,
        nc.sbuf_tensor("out_sb", [2, 2 * S], F32) as out_sb_t,
        nc.psum_tensor("psum_y", [2, S], F32) as psum_y_t,
        nc.psum_tensor("psum_z", [2, S], F32) as psum_z_t,
        nc.semaphore("s_w0") as s_w0,
        nc.semaphore("s_w1") as s_w1,
        nc.semaphore("s_w2") as s_w2,
        nc.semaphore("s_w3") as s_w3,
        nc.semaphore("s_w4") as s_w4,
        nc.semaphore("s_w5") as s_w5,
        nc.semaphore("s_v") as s_v,
        nc.semaphore("s_abs") as s_abs,
        nc.semaphore("s_pe") as s_pe,
        nc.semaphore("s_copy") as s_copy,
        nc.Block() as block,
    ):
        w_sb = w_sb_t.ap()
        a_sb = a_sb_t.ap()
        v_sb = v_sb_t.ap()
        out_sb = out_sb_t.ap()
        psum_y = psum_y_t.ap()
        psum_z = psum_z_t.ap()
        s_w = [s_w0, s_w1, s_w2, s_w3, s_w4, s_w5]

        def wsem(i):
            return s_w[i % NB], 16 * (i // NB + 1)

        @block.sync
        def _(sync):
            for i, ksup in enumerate(SUPS):
                if i >= NB:
                    # slot reuse: PE finished supertile i-NB (implies abs too)
                    sync.wait_ge(s_pe, i - NB + 1)
                sem, _ = wsem(i)
                sync.dma_start(
                    w_sb[:, i % NB, :ksup, :],
                    wt[:, starts[i]:starts[i] + ksup, :],
                ).then_inc(sem, 16)
                if i == 0:
                    sync.dma_start(v_sb[:], vecs).then_inc(s_v, 16)
            sync.wait_ge(s_copy, 2)
            # every DMA needs a sem update (walrus asserts otherwise)
            sync.dma_start(yz, out_sb[:]).then_inc(s_v, 16)

        @block.vector
        def _(vector):
            for i, ksup in enumerate(SUPS):
                sem, val = wsem(i)
                vector.wait_ge(sem, val)
                if i >= NB:
                    vector.wait_ge(s_pe, i - NB + 1)
                # two abs chunks per supertile so Z matmuls start earlier
                h = max(1, ksup // 2)
                for c0, c1 in ((0, h), (h, ksup)):
                    if c1 > c0:
                        vector.tensor_scalar(
                            a_sb[:, i % NB, c0:c1, :].bitcast(I16),
                            w_sb[:, i % NB, c0:c1, :].bitcast(I16),
                            0x7FFF, None, mybir.AluOpType.bitwise_and,
                        ).then_inc(s_abs, 1)
                    else:
                        vector.nop().then_inc(s_abs, 1)
            vector.wait_ge(s_pe, nsup)
            vector.tensor_copy(out=out_sb[:, 0:S], in_=psum_y[:])
            vector.tensor_copy(
                out=out_sb[:, S:2 * S], in_=psum_z[:]).then_inc(s_copy, 2)

        @block.tensor
        def _(tensor):
            tensor.wait_ge(s_v, 16)
            for i, ksup in enumerate(SUPS):
                sem, val = wsem(i)
                tensor.wait_ge(sem, val)
                h = max(1, ksup // 2)
                for j in range(ksup):
                    ki = starts[i] + j
                    tensor.matmul(
                        psum_y[:],
                        v_sb[:, ki, 0:2],
                        w_sb[:, i % NB, j, :],
                        start=(ki == 0), stop=(ki == KT - 1))
                    if j == 0:
                        tensor.wait_ge(s_abs, 2 * i + 1)
                    elif j == h:
                        tensor.wait_ge(s_abs, 2 * i + 2)
                    mm = tensor.matmul(
                        psum_z[:],
                        v_sb[:, ki, 2:4],
                        a_sb[:, i % NB, j, :],
                        start=(ki == 0), stop=(ki == KT - 1))
                mm.then_inc(s_pe, 1)

    nc.compile()
    return nc


def _build_nc():
    nc = bacc.Bacc("TRN2", target_bir_lowering=False, debug=False,
                   num_devices=NCORES)
    # partition-major: wt[p, ki, n] = W^T[ki*128 + p, n]
    wt = nc.dram_tensor("wt", [P, KT, S], F16, kind="ExternalInput").ap()
    vecs = nc.dram_tensor("vecs", [P, KT, 4], F16, kind="ExternalInput").ap()
    yz = nc.dram_tensor("yz", [2, 2 * S], F32, kind="ExternalOutput").ap()

    # first supertiles are small so the PE starts early; rest amortize DMA
    sups = [2, 6] + [KSUP] * ((KT - 8) // KSUP)
    assert sum(sups) == KT

    with tile.TileContext(nc) as tc:
        with (
            tc.tile_pool(name="wp", bufs=5) as wpool,
            tc.tile_pool(name="apool", bufs=5) as apool,
            tc.tile_pool(name="vp", bufs=1) as vpool,
            tc.tile_pool(name="op", bufs=1) as opool,
            tc.tile_pool(name="ps", bufs=1, space="PSUM") as pspool,
            tc.tile_pool(name="wu", bufs=1) as wupool,
            tc.tile_pool(name="wups", bufs=1, space="PSUM") as wupspool,
        ):
            # Warm-up: ~9 dummy matmuls on scratch data run while the first
            # weight DMAs are in flight, flipping the PE HAM clock gate from
            # 1.2 to 2.4 GHz (~3.4us of sustained PE activity) before the
            # real matmul stream starts.
            wu_sb = wupool.tile([P, S], F16)
            nc.vector.memset(wu_sb[:], 0.0)
            wu_ps = wupspool.tile([2, S], F32)
            for _ in range(13):
                nc.tensor.matmul(
                    wu_ps[:], wu_sb[:, 0:2], wu_sb[:],
                    start=True, stop=True)

            v_sb = vpool.tile([P, KT, 4], F16)
            nc.sync.dma_start(v_sb[:], vecs)

            psum_y = pspool.tile([2, S], F32, tag="Y")
            # Z accumulates at PSUM base partition 32 so its matmuls land on
            # a different PE column-group and run concurrently with Y's.
            psum_z = pspool.tile([34, S], F32, tag="Z", name="psum_z")[32:34]

            ki = 0
            for si, ksup in enumerate(sups):
                w_sb = wpool.tile([P, KSUP, S], F16, tag="w", name="w_sb")[:, :ksup, :]
                # alternate the two HWDGE rings (SP / ACT sequencers)
                dma_eng = nc.sync if si % 2 == 0 else nc.scalar
                dma_eng.dma_start(w_sb[:], wt[:, ki:ki + ksup, :])
                a_sb = apool.tile([P, KSUP, S], F16, tag="a", name="a_sb")[:, :ksup, :]
                # fp16 abs = clear the sign bit (abs_max isn't a valid
                # TensorScalar ALU op)
                nc.vector.tensor_scalar(
                    a_sb.bitcast(mybir.dt.int16), w_sb.bitcast(mybir.dt.int16),
                    0x7FFF, None, mybir.AluOpType.bitwise_and)
                for j in range(ksup):
                    nc.tensor.matmul(
                        psum_y[:],
                        v_sb[:, ki, 0:2],
                        w_sb[:, j, :],
                        start=(ki == 0), stop=(ki == KT - 1),
                        tile_position=(0, 0))
                    nc.tensor.matmul(
                        psum_z[:],
                        v_sb[:, ki, 2:4],
                        a_sb[:, j, :],
                        start=(ki == 0), stop=(ki == KT - 1),
                        tile_position=(0, 32))
                    ki += 1

            # pack along free dim: row0 = [y1 | z2], row1 = [ya | zc]
            out_sb = opool.tile([2, 2 * S], F32)
            nc.scalar.copy(out=out_sb[:, 0:S], in_=psum_y[:])
            nc.vector.tensor_copy(out=out_sb[:, S:2 * S], in_=psum_z[:])
            nc.sync.dma_start(yz, out_sb[:])

    nc.compile()
    return nc


USE_RAW = False


def get_nc():
    if "nc" not in _CACHE:
        _CACHE["nc"] = _build_nc_raw() if USE_RAW else _build_nc()
    return _CACHE["nc"]


def _host_vectors(bounds, bounds0, beta, lmbda, mu):
    l, u = bounds[0].astype(np.float64), bounds[1].astype(np.float64)
    l0, u0 = bounds0[0].astype(np.float64), bounds0[1].astype(np.float64)
    beta = beta.astype(np.float64)
    lmbda = lmbda.astype(np.float64)
    mu = mu.astype(np.float64)
    s1 = (beta + lmbda) / 2
    s2 = (beta - lmbda) / 2
    v1 = (l + u) / 2
    v2 = (l - u) / 2
    a = (s1 * (l0 + u0) + s2 * (l0 - u0) + mu) / 2
    c = (s2 * (l0 + u0) + s1 * (l0 - u0) - mu) / 2
    vecs = np.stack([v1, a, v2, c], axis=1)              # [N_IN, 4]
    return np.ascontiguousarray(
        vecs.reshape(KT, P, 4).transpose(1, 0, 2)).astype(np.float16)


def build_in_maps(weight, bounds, bounds0, beta, lmbda, mu):
    vecs = _host_vectors(bounds, bounds0, beta, lmbda, mu)
    in_maps = []
    for i in range(NCORES):
        wt = weight[i * S:(i + 1) * S].T                 # [N_IN, S] view
        wt = np.ascontiguousarray(
            wt.reshape(KT, P, S).transpose(1, 0, 2)).astype(np.float16)
        in_maps.append({"wt": wt, "vecs": vecs})
    return in_maps


def kernel(weight, bias, bounds, bounds0, beta, lmbda, mu):
    nc = get_nc()
    in_maps = build_in_maps(weight, bounds, bounds0, beta, lmbda, mu)
    res = run_bass_kernel_spmd(nc, in_maps, core_ids=list(range(NCORES)))

    lower = np.empty(N_OUT, np.float32)
    upper = np.empty(N_OUT, np.float32)
    for i in range(NCORES):
        yz = res.results[i]["yz"]
        y1, z2 = yz[0, :S], yz[0, S:]
        ya, zc = yz[1, :S], yz[1, S:]
        b = bias[i * S:(i + 1) * S]
        lower[i * S:(i + 1) * S] = np.maximum(y1 + z2, ya + zc) + b
        upper[i * S:(i + 1) * S] = np.minimum(y1 - z2, ya - zc) + b
    return np.stack([lower, upper], axis=0)
